# revision 2
# baseline (speedup 1.0000x reference)
"""Trainium2 Bass kernel v4 for nn_AttentionSubLayer — all-bf16.

Sharding (as v1-v3): 8 cores = 2 batch groups x 4-way sequence parallel with
causal load balancing.  Core c handles batch c//4 and query blocks {p, 7-p}
(256 tokens each, p = c%4).  K/V projected on the owning quarter and
AllGathered (bf16) within each 4-core batch group.

v4 changes vs v3 (686 us, NaN from the integer Quake seed):
 - rsqrt seed via value-CAST of the float bits + scalar-engine Exp
   (exp(bits(x) * -ln2/2^24 + B) ~= x^-1/2, 4.6% max err), then 2 Newton
   steps on DVE.  No integer ALU arithmetic, no Ln -> still one ACT table
   set for the whole kernel.
 - xq/xk/xv supplied pre-transposed by the host: all 48 xbar-transpose DMAs
   (~1.25 us each, serialized on the sync queue) become 12 large plain DMAs.
 - K stored transposed ([kv*hd, t]) from phase 1 (PE transpose, idle there)
   so phase-3 K loads are plain DMAs too.
 - rms sum-of-squares via ACT Square+accum_out (Square shares the exp table
   set) instead of DVE mult + slow 1x tensor_reduce.
 - phase-1 weight loads moved to the scalar HWDGE queue (x chunks on sync).
 - psb double-buffered (psy single) so the per-kv-group gate chain pipelines
   instead of stalling the PE queue ~5 us per kv head (HAM re-throttle).
"""

import math
import sys
import types
from contextlib import ExitStack

sys.path.insert(0, "/opt/trn_rl_repo")

import numpy as np

# ---------------------------------------------------------------- problem dims
B, T, D, H, KVH, HD = 2, 2048, 2048, 16, 4, 128
N_LAYER = 24
EPS = 1e-8
NCORE = 8
TB = 256          # token block for attention tiling
NBLK = T // TB    # 8 blocks per batch
QTOK = 2 * TB     # 512 q tokens per core
KVTOK = 2 * TB    # 512 kv tokens per core (contiguous quarter)
INV_SQRT_HD = 1.0 / math.sqrt(HD)
OUT_SCALE = 2 * N_LAYER  # final rms divided by sqrt(2*N_LAYER)
LN2 = math.log(2.0)
RSQRT_A = -LN2 / (1 << 24)
RSQRT_B = 0.5 * (127.0 + 0.0450466) * LN2


def _install_ntff_hook():
    try:
        import antenv
        if "antenv.axon_hooks" in sys.modules:
            return
        from trn_agent_boot.trn_boot import _ntff_profile_via_ctypes
        hook = _ntff_profile_via_ctypes("/opt/axon/libaxon_pjrt.so")
        mod = types.ModuleType("antenv.axon_hooks")
        mod.get_axon_ntff_profile_hook = lambda: hook
        antenv.axon_hooks = mod
        sys.modules["antenv.axon_hooks"] = mod
    except Exception:
        pass


_CACHE = {}


def _build():
    if "nc" in _CACHE:
        return _CACHE["nc"]

    import concourse.bass as bass
    import concourse.mybir as mybir
    import concourse.tile as tile
    from concourse import bacc
    from concourse.masks import make_identity

    f32 = mybir.dt.float32
    u32 = mybir.dt.uint32
    bf16 = mybir.dt.bfloat16
    AF = mybir.ActivationFunctionType
    ALU = mybir.AluOpType

    nc = bacc.Bacc("TRN2", target_bir_lowering=False, debug=False,
                   num_devices=NCORE)

    # ------------------------------------------------------------- I/O tensors
    xqT_in = nc.dram_tensor("xqT_in", [128, 16, QTOK], bf16,
                            kind="ExternalInput")
    xkT_in = nc.dram_tensor("xkT_in", [128, 16, KVTOK + 128], bf16,
                            kind="ExternalInput")
    xvT_in = nc.dram_tensor("xvT_in", [128, 16, KVTOK + 128], bf16,
                            kind="ExternalInput")
    Wq = nc.dram_tensor("Wq", [D, H * HD], bf16, kind="ExternalInput")
    Wg = nc.dram_tensor("Wg", [D, H * HD], bf16, kind="ExternalInput")
    Wo = nc.dram_tensor("Wo", [H * HD, D], bf16, kind="ExternalInput")
    Wk12 = nc.dram_tensor("Wk12", [D, 2 * KVH * HD], bf16, kind="ExternalInput")
    Wv12 = nc.dram_tensor("Wv12", [D, 2 * KVH * HD], bf16, kind="ExternalInput")
    cos_q = nc.dram_tensor("cos_q", [QTOK, HD], bf16, kind="ExternalInput")
    sin_q = nc.dram_tensor("sin_q", [QTOK, HD], bf16, kind="ExternalInput")
    cos_k = nc.dram_tensor("cos_k", [KVTOK, HD], bf16, kind="ExternalInput")
    sin_k = nc.dram_tensor("sin_k", [KVTOK, HD], bf16, kind="ExternalInput")
    # multiplicative {0,1} masks, layout [tk, slot, (ss, tq)]
    mask_all = nc.dram_tensor("mask_all", [128, 12, 2 * TB], bf16,
                              kind="ExternalInput")
    bsel_in = nc.dram_tensor("bsel_in", [8, 8, 128], bf16, kind="ExternalInput")
    out_y = nc.dram_tensor("out_y", [QTOK, D], f32, kind="ExternalOutput")

    # staging for K/V allgathers (within 4-core batch group)
    # K transposed [(kv,hd), t]; V natural [t, (kv,hd)]
    k_loc = nc.dram_tensor("k_loc", [KVH * HD, KVTOK], bf16)
    v_loc = nc.dram_tensor("v_loc", [KVTOK, KVH * HD], bf16)
    k_gath = nc.dram_tensor("k_gath", [4, KVH * HD, KVTOK], bf16)
    v_gath = nc.dram_tensor("v_gath", [4, KVTOK, KVH * HD], bf16)

    with tile.TileContext(nc) as tc, ExitStack() as es:
        # ------------------------------------------------------------ constants
        cpool = es.enter_context(tc.tile_pool(name="consts", bufs=1))
        ident = cpool.tile([128, 128], bf16)
        make_identity(nc, ident[:])
        bsel = cpool.tile([8, 8, 128], bf16)
        nc.scalar.dma_start(out=bsel[:], in_=bsel_in[:])
        e_strip = cpool.tile([128, 15], bf16)
        nc.vector.memset(e_strip[:], 0.0)
        nc.vector.memset(e_strip[:, 7:8], 1.0)
        cosq_sb = cpool.tile([128, 4, HD], bf16)
        sinq_sb = cpool.tile([128, 4, HD], bf16)
        cosk_sb = cpool.tile([128, 4, HD], bf16)
        sink_sb = cpool.tile([128, 4, HD], bf16)
        for dst, src in ((cosq_sb, cos_q), (sinq_sb, sin_q),
                         (cosk_sb, cos_k), (sink_sb, sin_k)):
            nc.scalar.dma_start(
                out=dst[:], in_=src[:].rearrange("(a p) c -> p a c", a=4))

        # ============================================================ helpers
        def rsqrt_dve(dst, src, pool, scale=1.0, bias=0.0, tag="rq", steps=2):
            """dst = rsqrt(src*scale + bias); src f32 (SBUF or PSUM).

            Seed: exp(float(bits(x)) * -ln2/2^24 + B) ~ x^-0.5 (<=4.7% err),
            then Newton steps (1 -> 3.4e-4, 2 -> 2e-5).  Uses only Exp on ACT
            (same table set as the softmax exp) and DVE arithmetic.
            """
            shp = list(src.shape)
            if scale != 1.0 or bias != 0.0:
                m = pool.tile(shp, f32, tag=tag + "m")
                nc.vector.tensor_scalar(out=m[:], in0=src, scalar1=float(scale),
                                        scalar2=float(bias), op0=ALU.mult,
                                        op1=ALU.add)
                x = m[:]
            else:
                x = src
            cf = pool.tile(shp, f32, tag=tag + "c")
            nc.vector.tensor_copy(out=cf[:], in_=x.bitcast(u32))
            r = pool.tile(shp, f32, tag=tag + "r")
            bias_t = pool.tile([shp[0], 1], f32, tag=tag + "b")
            nc.vector.memset(bias_t[:], RSQRT_B)
            nc.scalar.activation(out=r[:], in_=cf[:], func=AF.Exp,
                                 scale=RSQRT_A, bias=bias_t[:])
            t = pool.tile(shp, f32, tag=tag + "t")
            for it in range(steps):
                nc.vector.tensor_tensor(out=t[:], in0=r[:], in1=r[:],
                                        op=ALU.mult)
                nc.vector.tensor_tensor(out=t[:], in0=x, in1=t[:], op=ALU.mult)
                nc.vector.tensor_scalar(out=t[:], in0=t[:], scalar1=-0.5,
                                        scalar2=1.5, op0=ALU.mult, op1=ALU.add)
                nc.vector.tensor_tensor(out=(dst if it == steps - 1 else r[:]),
                                        in0=r[:], in1=t[:], op=ALU.mult)

        def rope(dst_t, src_t, nh, cos_sb, sin_sb, m, smp):
            """dst = rope(src); all bf16; cos/sin tiles [128,4,HD]."""
            half = HD // 2
            cos_t = cos_sb[:, m, :]
            sin_lo = sin_sb[:, m, 0:half]
            sin_hi = sin_sb[:, m, half:HD]
            t1 = smp.tile([128, half], bf16, tag="ro1")
            for h in range(nh):
                d = dst_t[:, 128 * h:128 * h + 128]
                s = src_t[:, 128 * h:128 * h + 128]
                d_lo = dst_t[:, 128 * h:128 * h + half]
                d_hi = dst_t[:, 128 * h + half:128 * h + 128]
                s_lo = src_t[:, 128 * h:128 * h + half]
                s_hi = src_t[:, 128 * h + half:128 * h + 128]
                nc.vector.tensor_tensor(out=d, in0=s, in1=cos_t, op=ALU.mult)
                nc.vector.tensor_tensor(out=t1[:], in0=s_hi, in1=sin_lo, op=ALU.mult)
                nc.vector.tensor_tensor(out=d_lo, in0=d_lo, in1=t1[:], op=ALU.subtract)
                nc.vector.tensor_tensor(out=t1[:], in0=s_lo, in1=sin_hi, op=ALU.mult)
                nc.vector.tensor_tensor(out=d_hi, in0=d_hi, in1=t1[:], op=ALU.add)

        # ===================================================== phase 1: K / V
        p_xqT = es.enter_context(tc.tile_pool(name="ppxqT", bufs=1))
        xqT = p_xqT.tile([128, 16, QTOK], bf16, tag="xqT", name="xqT")
        # one persistent weight pool for all phases: phase-boundary SBUF
        # reuse otherwise chains phase-2 weight prefetch onto phase-1 tails
        wpool = es.enter_context(tc.tile_pool(name="wpool", bufs=4))

        ag = {}
        with tc.tile_pool(name="p1xt", bufs=2) as xtp, \
             tc.tile_pool(name="p1kv", bufs=3) as kvp, \
             tc.tile_pool(name="p1ps", bufs=1, space="PSUM") as pskv, \
             tc.tile_pool(name="p1pt", bufs=2, space="PSUM") as pstp, \
             tc.tile_pool(name="p1sm", bufs=3) as smp:
            for (xT_dram, W12, loc, is_k) in ((xkT_in, Wk12, k_loc, True),
                                              (xvT_in, Wv12, v_loc, False)):
                xT = xtp.tile([128, 16, KVTOK + 128], bf16, tag="xT",
                              name="xkT" if is_k else "xvT")
                for c8 in range(8):
                    nc.sync.dma_start(out=xT[:, 2 * c8:2 * c8 + 2, :],
                                      in_=xT_dram[:, 2 * c8:2 * c8 + 2, :])
                pkvs = [pskv.tile([128, KVH * HD], f32, tag=f"pkv{m}",
                                  name=f"pkv{m}") for m in range(4)]
                for k in range(16):
                    w12 = wpool.tile([128, 2048], bf16, tag="w")
                    nc.scalar.dma_start(out=w12[:, 0:1024],
                                        in_=W12[128 * k:128 * k + 128, :])
                    for m in range(4):
                        nc.tensor.matmul(pkvs[m][:],
                                         xT[:, k, 128 + 128 * m:256 + 128 * m],
                                         w12[:, 0:512],
                                         start=(k == 0), stop=False)
                        nc.tensor.matmul(pkvs[m][:],
                                         xT[:, k, 127 + 128 * m:255 + 128 * m],
                                         w12[:, 512:1024],
                                         start=False, stop=(k == 15))
                stage = []
                for m in range(4):
                    pkv = pkvs[m]
                    nat_f = kvp.tile([128, KVH * HD], bf16, tag="natf")
                    nc.scalar.copy(out=nat_f[:], in_=pkv[:])
                    s2 = smp.tile([128, KVH], f32, tag="rs2")
                    scrap = smp.tile([128, HD], f32, tag="rscrap")
                    for h in range(KVH):
                        nc.scalar.activation(out=scrap[:],
                                             in_=pkv[:, 128 * h:128 * h + 128],
                                             func=AF.Square,
                                             accum_out=s2[:, h:h + 1])
                    ri = smp.tile([128, KVH], f32, tag="rri")
                    rsqrt_dve(ri[:], s2[:], smp, scale=1.0 / HD, bias=EPS,
                              tag="rk")
                    nat = kvp.tile([128, KVH * HD], bf16, tag="nat")
                    for h in range(KVH):
                        nc.vector.tensor_scalar_mul(nat[:, 128 * h:128 * h + 128],
                                                    nat_f[:, 128 * h:128 * h + 128],
                                                    ri[:, h:h + 1])
                    if is_k:
                        rot = kvp.tile([128, KVH * HD], bf16, tag="rot")
                        rope(rot, nat, KVH, cosk_sb, sink_sb, m, smp)
                        ktT = kvp.tile([128, KVH, 128], bf16, tag="ktT")
                        for h in range(KVH):
                            pst = pstp.tile([128, 128], bf16, tag="pst")
                            nc.tensor.transpose(pst[:],
                                                rot[:, 128 * h:128 * h + 128],
                                                ident[:])
                            nc.vector.tensor_copy(out=ktT[:, h, :], in_=pst[:])
                        d = nc.gpsimd.dma_start(
                            out=k_loc[:, 128 * m:128 * m + 128]
                            .rearrange("(a p) c -> p a c", a=KVH),
                            in_=ktT[:])
                    else:
                        d = nc.gpsimd.dma_start(
                            out=v_loc[128 * m:128 * m + 128, :], in_=nat[:])
                    stage.append(d)
                gname = "k" if is_k else "v"
                ag[gname] = nc.gpsimd.collective_compute(
                    "AllGather", ALU.bypass,
                    replica_groups=[[0, 1, 2, 3], [4, 5, 6, 7]],
                    ins=[(k_loc if is_k else v_loc)[:]],
                    outs=[(k_gath if is_k else v_gath)[:]])
                for d in stage:
                    tile.add_dep_helper(ag[gname].ins, d.ins,
                                        reason="stage before allgather")
                if is_k:
                    # prefetch xqT early (sync queue, after xk chunks)
                    for c4 in range(4):
                        nc.sync.dma_start(out=xqT[:, 4 * c4:4 * c4 + 4, :],
                                          in_=xqT_in[:, 4 * c4:4 * c4 + 4, :])

        # ===================================================== phase 2: Q / G
        p_gT = es.enter_context(tc.tile_pool(name="ppgT", bufs=1))
        gT_sb = p_gT.tile([128, H, QTOK], bf16, tag="gT", name="gT_sb")
        p_qT = es.enter_context(tc.tile_pool(name="ppqT", bufs=1))
        qT_sb = p_qT.tile([128, H, QTOK], bf16, tag="qT", name="qT_sb")
        with tc.tile_pool(name="p2q", bufs=1) as qp, \
             tc.tile_pool(name="p2rot", bufs=4) as rotp, \
             tc.tile_pool(name="p2ps", bufs=1, space="PSUM") as psq, \
             tc.tile_pool(name="p2sm", bufs=2) as smp:
            # G projection -> transposed [gcol, tok] directly, 8 banks at a time
            for gqp in range(2):
                psg = [psq.tile([128, 512], f32, tag=f"pp{i}", name=f"pg{i}")
                       for i in range(8)]
                for k in range(16):
                    wgt = wpool.tile([128, 2048], bf16, tag="w")
                    nc.sync.dma_start(out=wgt[:, 0:1024],
                                      in_=Wg[128 * k:128 * k + 128,
                                             1024 * gqp:1024 * gqp + 1024])
                    for gi in range(8):
                        nc.tensor.matmul(
                            psg[gi][:],
                            wgt[:, 128 * gi:128 * gi + 128],
                            xqT[:, k, :],
                            start=(k == 0), stop=(k == 15))
                for gi in range(8):
                    nc.vector.tensor_copy(out=gT_sb[:, 8 * gqp + gi, :],
                                          in_=psg[gi][:])

            # Q projection in two m-passes (tokens 0-255, then 256-511) with
            # full-width weight rows; each pass's rms/rope (DVE) overlaps the
            # other pass's matmuls.  Wq is read twice -- cheap vs a PE stall.
            q_sb = [qp.tile([128, H * HD], bf16, tag=f"q{m}", name=f"q{m}")
                    for m in range(4)]
            rots = [None] * 4

            def q_tail_dve(m):
                s2 = smp.tile([128, H], f32, tag="qs2")
                scrap = smp.tile([128, HD], f32, tag="qscrap")
                for h in range(H):
                    nc.scalar.activation(out=scrap[:],
                                         in_=q_sb[m][:, 128 * h:128 * h + 128],
                                         func=AF.Square,
                                         accum_out=s2[:, h:h + 1])
                # rsqrt(s2/HD + eps)/sqrt(HD) == rsqrt(s2 + HD*eps) for HD=128
                ri = smp.tile([128, H], f32, tag="qri")
                rsqrt_dve(ri[:], s2[:], smp, scale=1.0, bias=HD * EPS, tag="rq2")
                for h in range(H):
                    sl = q_sb[m][:, 128 * h:128 * h + 128]
                    nc.vector.tensor_scalar_mul(sl, sl, ri[:, h:h + 1])
                rot = rotp.tile([128, H * HD], bf16, tag="qrot",
                                name=f"rot{m}")
                rope(rot, q_sb[m], H, cosq_sb, sinq_sb, m, smp)
                rots[m] = rot

            for mp in range(2):
                ps = [psq.tile([128, 512], f32, tag=f"pp{i}", name=f"pq{i}")
                      for i in range(8)]
                for k in range(16):
                    wqt = wpool.tile([128, 2048], bf16, tag="w")
                    nc.sync.dma_start(out=wqt[:],
                                      in_=Wq[128 * k:128 * k + 128, :])
                    for mm in range(2):
                        for n in range(4):
                            nc.tensor.matmul(ps[4 * mm + n][:],
                                             xqT[:, k, 256 * mp + 128 * mm:
                                                 256 * mp + 128 * mm + 128],
                                             wqt[:, 512 * n:512 * n + 512],
                                             start=(k == 0), stop=(k == 15))
                for mm in range(2):
                    for n in range(4):
                        nc.scalar.copy(
                            out=q_sb[2 * mp + mm][:, 512 * n:512 * n + 512],
                            in_=ps[4 * mm + n][:])
                q_tail_dve(2 * mp)
                q_tail_dve(2 * mp + 1)

            for m in range(4):
                for h in range(H):
                    # reuse two projection-PSUM slots (free after Q evac)
                    pst = psq.tile([128, 128], bf16, tag=f"pp{h % 2}",
                                   name="pst")
                    nc.tensor.transpose(pst[:],
                                        rots[m][:, 128 * h:128 * h + 128],
                                        ident[:])
                    nc.vector.tensor_copy(out=qT_sb[:, h, 128 * m:128 * m + 128],
                                          in_=pst[:])

        # ==================================================== phase 3: attention
        p_gTr = es.enter_context(tc.tile_pool(name="ppgTr", bufs=1))
        gTr_sb = p_gTr.tile([128, H, QTOK], bf16, tag="gTr", name="gTr_sb")
        with tc.tile_pool(name="p3m", bufs=1) as mp, \
             tc.tile_pool(name="p3kv", bufs=2) as kvp, \
             tc.tile_pool(name="p3pt", bufs=3) as ptq, \
             tc.tile_pool(name="p3ps", bufs=2, space="PSUM") as pss_p, \
             tc.tile_pool(name="p3py", bufs=1, space="PSUM") as psy_p, \
             tc.tile_pool(name="p3pn", bufs=1, space="PSUM") as psn_p, \
             tc.tile_pool(name="p3pb", bufs=2, space="PSUM") as psb_p, \
             tc.tile_pool(name="p3ys", bufs=2) as ysp, \
             tc.tile_pool(name="p3sm", bufs=4) as smp:
            masks_sb = mp.tile([128, 12, 2 * TB], bf16, tag="masks")
            nc.scalar.dma_start(out=masks_sb[:], in_=mask_all[:])

            kload = {"k": [], "v": []}
            for kv in range(KVH):
                K_sb = kvp.tile([128, NBLK, TB], bf16, tag="K")
                V_sb = kvp.tile([128, 2 * NBLK, HD], bf16, tag="V")
                for r in range(4):
                    d = nc.sync.dma_start(
                        out=K_sb[:, 2 * r:2 * r + 2, :],
                        in_=k_gath[r, 128 * kv:128 * kv + 128, :])
                    kload["k"].append(d)
                    d = nc.sync.dma_start(
                        out=V_sb[:, 4 * r:4 * r + 4, :],
                        in_=v_gath[r, :, 128 * kv:128 * kv + 128]
                        .rearrange("(a p) c -> p a c", a=4))
                    kload["v"].append(d)
                psn = psn_p.tile([8, TB], f32, tag="psn")
                ysb = ysp.tile([128, 8, TB], bf16, tag="ysb")
                for hi in range(4):
                    h = 4 * kv + hi
                    for s01, nblk in ((0, 4), (1, NBLK)):
                        r = 2 * hi + s01
                        psy = psy_p.tile([128, TB], f32, tag="psy")
                        for g in range(nblk // 2):
                            pss = pss_p.tile([128, 4 * TB], f32, tag="pss")
                            for ii in range(2):
                                i = 2 * g + ii
                                for ss in range(2):
                                    nc.tensor.matmul(
                                        pss[:, 512 * ii + TB * ss:
                                            512 * ii + TB * ss + TB],
                                        K_sb[:, i, 128 * ss:128 * ss + 128],
                                        qT_sb[:, h, TB * s01:TB * s01 + TB],
                                        start=True, stop=True)
                            pt = ptq.tile([128, 4 * TB], bf16, tag="pt")
                            nc.scalar.activation(out=pt[:], in_=pss[:],
                                                 func=AF.Exp)
                            # mask only groups that can contain diagonal or
                            # invalid blocks (s01=0: all; s01=1: blocks 4..7)
                            if s01 == 0 or g >= 2:
                                slot0 = 2 * g if s01 == 0 else 4 + 2 * g
                                msk = masks_sb[:, slot0:slot0 + 2, :]
                                nc.vector.tensor_tensor(
                                    out=pt[:],
                                    in0=pt[:],
                                    in1=msk.rearrange("p a b -> p (a b)"),
                                    op=ALU.mult)
                            for ii in range(2):
                                i = 2 * g + ii
                                for ss in range(2):
                                    nc.tensor.matmul(
                                        psy[:], V_sb[:, 2 * i + ss, :],
                                        pt[:, 512 * ii + TB * ss:
                                           512 * ii + TB * ss + TB],
                                        start=(g == 0 and ii == 0 and ss == 0),
                                        stop=(g == nblk // 2 - 1 and ii == 1
                                              and ss == 1))
                        # l2 norm (cancels softmax denom): accumulate per-head
                        # sq sums into psn row r; gate with g now, scale later.
                        psy_f = smp.tile([128, TB], bf16, tag="psyf")
                        nc.vector.tensor_copy(out=psy_f[:], in_=psy[:])
                        ysq = smp.tile([128, TB], bf16, tag="ysq")
                        nc.vector.tensor_tensor(out=ysq[:], in0=psy_f[:],
                                                in1=psy_f[:], op=ALU.mult)
                        nc.tensor.matmul(psn[:], e_strip[:, 7 - r:15 - r],
                                         ysq[:], start=(r == 0), stop=(r == 7))
                        nc.vector.tensor_tensor(out=ysb[:, r, :], in0=psy_f[:],
                                                in1=gT_sb[:, h,
                                                          TB * s01:TB * s01 + TB],
                                                op=ALU.mult)
                rsc = smp.tile([8, TB], bf16, tag="rsc")
                rsqrt_dve(rsc[:], psn[:], smp, tag="rn", steps=1)
                for hi in range(4):
                    h = 4 * kv + hi
                    for s01 in range(2):
                        r = 2 * hi + s01
                        psb = psb_p.tile([128, TB], f32, tag="psb")
                        nc.tensor.matmul(psb[:], bsel[:, r, :], rsc[:],
                                         start=True, stop=True)
                        nc.vector.tensor_tensor(
                            out=gTr_sb[:, h, TB * s01:TB * s01 + TB],
                            in0=ysb[:, r, :], in1=psb[:], op=ALU.mult)
            for d in kload["k"]:
                tile.add_dep_helper(d.ins, ag["k"].ins, reason="ag_k first")
            for d in kload["v"]:
                tile.add_dep_helper(d.ins, ag["v"].ins, reason="ag_v first")

        # ==================================================== phase 4: out proj
        with tc.tile_pool(name="p4o", bufs=1) as op_, \
             tc.tile_pool(name="p4ps", bufs=1, space="PSUM") as pso_p, \
             tc.tile_pool(name="p4sm", bufs=2) as smp:
            out_sb = [op_.tile([128, D], f32, tag=f"o{m}", name=f"o{m}")
                      for m in range(4)]
            for np_ in range(2):
                pso = [pso_p.tile([128, 512], f32, tag=f"po{i}", name=f"po{i}")
                       for i in range(8)]
                for k in range(16):
                    wot = wpool.tile([128, 2048], bf16, tag="w")
                    nc.sync.dma_start(out=wot[:, 0:1024],
                                      in_=Wo[128 * k:128 * k + 128,
                                             1024 * np_:1024 * np_ + 1024])
                    for m in range(4):
                        for nn in range(2):
                            nc.tensor.matmul(pso[2 * m + nn][:],
                                             gTr_sb[:, k, 128 * m:128 * m + 128],
                                             wot[:, 512 * nn:512 * nn + 512],
                                             start=(k == 0), stop=(k == 15))
                for m in range(4):
                    for nn in range(2):
                        nc.scalar.copy(
                            out=out_sb[m][:, 1024 * np_ + 512 * nn:
                                          1024 * np_ + 512 * nn + 512],
                            in_=pso[2 * m + nn][:])
            for m in range(4):
                s2 = smp.tile([128, 1], f32, tag="os2")
                scrap = smp.tile([128, D], f32, tag="oscrap")
                nc.scalar.activation(out=scrap[:], in_=out_sb[m][:],
                                     func=AF.Square, accum_out=s2[:])
                r2 = smp.tile([128, 1], f32, tag="or2")
                rsqrt_dve(r2[:], s2[:], smp, scale=float(OUT_SCALE) / D,
                          bias=float(OUT_SCALE) * EPS, tag="ro")
                nc.vector.tensor_scalar_mul(out_sb[m][:], out_sb[m][:], r2[:])
                nc.sync.dma_start(out=out_y[128 * m:128 * m + 128, :],
                                  in_=out_sb[m][:])

    nc.compile()
    _CACHE["nc"] = nc
    return nc


def _host_inputs(xq, xk, xv, Wq, Wk, Wv, Wg, Wo, mix_k, mix_v):
    """Build the 8 per-core input maps (bf16, x pre-transposed)."""
    import ml_dtypes
    f = np.float32
    bf = ml_dtypes.bfloat16
    xq = np.asarray(xq, f)
    xk = np.asarray(xk, f)
    xv = np.asarray(xv, f)
    Wq_b = np.ascontiguousarray(np.asarray(Wq, f)).astype(bf)
    Wg_b = np.ascontiguousarray(np.asarray(Wg, f)).astype(bf)
    Wo_b = np.ascontiguousarray(np.asarray(Wo, f)).astype(bf)
    Wk = np.asarray(Wk, f)
    Wv = np.asarray(Wv, f)
    mix_k = np.asarray(mix_k, f)
    mix_v = np.asarray(mix_v, f)

    Wk12 = np.ascontiguousarray(np.concatenate(
        [(1.0 - mix_k)[:, None] * Wk, mix_k[:, None] * Wk], axis=1)).astype(bf)
    Wv12 = np.ascontiguousarray(np.concatenate(
        [(1.0 - mix_v)[:, None] * Wv, mix_v[:, None] * Wv], axis=1)).astype(bf)

    half = HD // 2
    inv_freq = 1.0 / (10000.0 ** (np.arange(half, dtype=np.float64) / half))
    ang = np.arange(T, dtype=np.float64)[:, None] * inv_freq[None, :]
    cos_t = np.concatenate([np.cos(ang), np.cos(ang)], axis=-1).astype(f)
    sin_t = np.concatenate([np.sin(ang), np.sin(ang)], axis=-1).astype(f)

    # multiplicative {0,1} masks, layout [tk, (ss, tq)]
    ii = np.arange(128)[:, None]
    jj = np.arange(TB)[None, :]
    diag_mask = np.zeros((128, 2, TB), f)
    for ss in range(2):
        diag_mask[:, ss, :] = np.where(128 * ss + ii <= jj, 1.0, 0.0)
    diag_mask = diag_mask.reshape(128, 2 * TB)
    ones_m = np.ones((128, 2 * TB), f)
    zeros_m = np.zeros((128, 2 * TB), f)

    bsel_np = np.zeros((8, 8, 128), f)
    for r in range(8):
        bsel_np[r, r, :] = 1.0
    bsel_np = bsel_np.astype(bf)

    def xpose(a):
        # [rows, D] -> [128, 16, rows]
        rows = a.shape[0]
        return np.ascontiguousarray(
            a.reshape(rows, 16, 128).transpose(2, 1, 0))

    in_maps = []
    for c in range(NCORE):
        b, p = divmod(c, 4)
        jq0, jq1 = p, NBLK - 1 - p
        rows_q = np.concatenate([np.arange(TB * jq0, TB * jq0 + TB),
                                 np.arange(TB * jq1, TB * jq1 + TB)])
        t0 = KVTOK * p
        rows_kv = np.arange(t0, t0 + KVTOK)

        xq_s = np.ascontiguousarray(xq[b, rows_q, :])
        xk_s = np.zeros((KVTOK + 128, D), f)
        xv_s = np.zeros((KVTOK + 128, D), f)
        xk_s[128:] = xk[b, t0:t0 + KVTOK, :]
        xv_s[128:] = xv[b, t0:t0 + KVTOK, :]
        if p > 0:
            xk_s[127] = xk[b, t0 - 1, :]
            xv_s[127] = xv[b, t0 - 1, :]

        mask = np.empty((12, 128, 2 * TB), f)
        for i in range(4):
            mask[i] = diag_mask if i == jq0 else (ones_m if i < jq0 else zeros_m)
        for i in range(NBLK):
            mask[4 + i] = diag_mask if i == jq1 else (ones_m if i < jq1 else zeros_m)
        mask = np.ascontiguousarray(mask.transpose(1, 0, 2))  # [tk, slot, x]

        in_maps.append({
            "xqT_in": xpose(xq_s).astype(bf),
            "xkT_in": xpose(xk_s).astype(bf),
            "xvT_in": xpose(xv_s).astype(bf),
            "Wq": Wq_b, "Wg": Wg_b, "Wo": Wo_b,
            "Wk12": Wk12, "Wv12": Wv12,
            "cos_q": np.ascontiguousarray(cos_t[rows_q]).astype(bf),
            "sin_q": np.ascontiguousarray(sin_t[rows_q]).astype(bf),
            "cos_k": np.ascontiguousarray(cos_t[rows_kv]).astype(bf),
            "sin_k": np.ascontiguousarray(sin_t[rows_kv]).astype(bf),
            "mask_all": mask.astype(bf),
            "bsel_in": bsel_np,
        })
    return in_maps


def _run(in_maps, trace=False, tmpdir=None):
    _install_ntff_hook()
    from concourse.bass_utils import run_bass_kernel_spmd
    nc = _build()
    return run_bass_kernel_spmd(nc, in_maps, list(range(NCORE)),
                                trace=trace, tmpdir=tmpdir)


def kernel(xq, xk, xv, Wq, Wk, Wv, Wg, Wo, mix_k, mix_v,
           _trace=False, _tmpdir=None):
    in_maps = _host_inputs(xq, xk, xv, Wq, Wk, Wv, Wg, Wo, mix_k, mix_v)
    res = _run(in_maps, trace=_trace, tmpdir=_tmpdir)
    out = np.empty((B, T, D), np.float32)
    for c in range(NCORE):
        b, p = divmod(c, 4)
        jq0, jq1 = p, NBLK - 1 - p
        y = res.results[c]["out_y"]
        out[b, TB * jq0:TB * jq0 + TB, :] = y[:TB]
        out[b, TB * jq1:TB * jq1 + TB, :] = y[TB:]
    kernel._last_exec_ns = res.exec_time_ns
    return out
